# revision 44
# baseline (speedup 1.0000x reference)
import numpy as np
import ml_dtypes

import concourse.bass as bass
import concourse.mybir as mybir
import concourse.bacc as cbacc
import concourse.tile as tile
from concourse.bass_utils import run_bass_kernel_spmd

FP = mybir.dt.float32
BF = mybir.dt.bfloat16
HF = mybir.dt.float16
AF = mybir.ActivationFunctionType
AX = mybir.AxisListType
OP = mybir.AluOpType

# problem dims (hardcoded): B=4, T=1024, P=64, D=128 -> 4096 clouds over 8 cores
NCORES = 8
NCLOUD = 512          # clouds per core
P = 64
D = 128
NPT = NCLOUD * P      # 32768 points per core
CHUNK = 512           # points per chunk (8 clouds, 4 128-col tiles)
NCH = NPT // CHUNK    # 64 chunks
TOTAL_CLOUDS = 4096


def bcast(ap, n):
    """append a step-0 broadcast dim of size n to an AP"""
    return bass.AP(tensor=ap.tensor, offset=ap.offset, ap=list(ap.ap) + [[0, n]])


def row_pair(tile_ap, k, last_row, n):
    """AP over partitions {k, last_row} x [0, n) of a 2D SBUF tile"""
    base = tile_ap[k : k + 1, 0:n]
    return bass.AP(tensor=base.tensor, offset=base.offset,
                   ap=[[last_row - k, 2]] + list(base.ap)[1:])


def build_bass(repeat=1, skip_collective=False, extra_collectives=0):
    nc = cbacc.Bacc(trn_type="TRN2", num_devices=NCORES)

    posT_d = nc.declare_dram_parameter("posT", [D, NPT], BF, isOutput=False)
    ctxT_d = nc.declare_dram_parameter("ctxT", [D, NCLOUD], BF, isOutput=False)
    td8_d = nc.declare_dram_parameter("td8", [5, NPT // 4], BF, isOutput=False)
    nsq4_d = nc.declare_dram_parameter("nsq4", [4, NPT // 4], BF, isOutput=False)
    wbig_d = nc.declare_dram_parameter("wbig", [128, 416], BF, isOutput=False)
    wf32_d = nc.declare_dram_parameter("wf32", [128, 385], FP, isOutput=False)
    wsm_d = nc.declare_dram_parameter("wsm", [64, 1185], BF, isOutput=False)
    wfs_d = nc.declare_dram_parameter("wfs", [64, 3], FP, isOutput=False)
    out_d = nc.declare_dram_parameter("out", [128, NCH * 4], FP, isOutput=True)

    # collective bounce buffers (DRAM internal)
    cc_in = nc.dram_tensor("cc_in", [1, P], FP)
    cc_out = nc.dram_tensor("cc_out", [8, P], FP)

    with tile.TileContext(nc) as tc:
        with (
            tc.tile_pool(name="consts", bufs=1) as cpool,
            tc.tile_pool(name="persist", bufs=1) as ppool,
            tc.tile_pool(name="posbuf", bufs=3) as wpool,
            tc.tile_pool(name="mid", bufs=3) as xpool,
            tc.tile_pool(name="small", bufs=2) as spool,
            tc.tile_pool(name="psH", bufs=2, space="PSUM") as psH,
            tc.tile_pool(name="psG", bufs=2, space="PSUM") as psG,
            tc.tile_pool(name="psQ", bufs=2, space="PSUM") as psQ,
            tc.tile_pool(name="psZ", bufs=1, space="PSUM") as psZ,
            tc.tile_pool(name="psM", bufs=1, space="PSUM") as psM,
        ):
            # ---- consolidated constants / weights (5 DMAs on the Pool queue) ----
            ctxT = cpool.tile([128, NCLOUD], BF)
            nc.gpsimd.dma_start(out=ctxT[:, :], in_=ctxT_d[:, :])
            wbig = cpool.tile([128, 416], BF)
            nc.gpsimd.dma_start(out=wbig[:, :], in_=wbig_d[:, :])
            wsm = cpool.tile([64, 1185], BF)
            nc.gpsimd.dma_start(out=wsm[:, :], in_=wsm_d[:, :])
            wf32 = cpool.tile([128, 385], FP)
            nc.gpsimd.dma_start(out=wf32[:, :], in_=wf32_d[:, :])
            wfs = cpool.tile([64, 3], FP)
            nc.gpsimd.dma_start(out=wfs[:, :], in_=wfs_d[:, :])

            c1c = wbig[:, 0:128]
            c1p = wbig[:, 128:256]
            c2w = wbig[:, 256:320]
            h1p = wbig[:, 320:352]
            hw3rep = wbig[:, 352:416]
            nsqn = wf32[:, 0:256]
            hb2r = wf32[:, 256:320]
            fold_sb = wf32[:, 320:384]
            bconst = wf32[:, 384:385]
            bo8 = wsm[0:8, 0:512]
            h1de4 = wsm[0:5, 512:640].rearrange("p (a b) -> p a b", b=32)
            hw2 = wsm[0:32, 640:656]
            hw2b = wsm[32:64, 640:656]
            hb2row = wsm[0:1, 1169:1185]
            cw3b = wsm[0:64, 656:657]
            cb1r = wsm[0:1, 657:785]
            sel4 = wsm[0:4, 785:1041]
            ones128 = wsm[0:1, 1041:1169]
            cb2 = wfs[0:64, 0:1]
            one11 = wfs[0:1, 1:2]
            ones81 = wfs[0:8, 2:3]

            nsq4 = cpool.tile([4, NPT // 4], BF)
            td8 = cpool.tile([5, NPT // 4], BF)
            g1TeBufs = []
            for i in range(3):
                g1Tei = cpool.tile([64, 256], BF, name=f"g1Te{i}")
                g1TeBufs.append(g1Tei)

            y_nat = cpool.tile([128, 4, 128], BF)
            yT2 = cpool.tile([8, NCH * 128], BF)

            # persistent accumulators
            comb = ppool.tile([128, NCH * 4], FP)   # 0.3*ctx' + 0.2*hier'
            t1s = ppool.tile([128, NCH * 12], FP)   # -d2/2 of 3 NN per point

            posT4_cache = {}

            def emit_prologue():
                # first positions batch ASAP on the SP queue
                posT4_cache.clear()
                posT4_cache[0] = wpool.tile([128, 4, CHUNK], BF, tag="posT", name="posT4")
                nc.sync.dma_start(
                    out=posT4_cache[0][:, :, :].rearrange("p a b -> p (a b)"),
                    in_=posT_d[:, 0 : 4 * CHUNK],
                )
                # nsq4/td8 piecewise on the DVE queue (chunk 0 needs piece 0 only)
                npc = NPT // 16
                for i in range(4):
                    nc.gpsimd.dma_start(
                        out=nsq4[:, npc * i : npc * (i + 1)],
                        in_=nsq4_d[:, npc * i : npc * (i + 1)],
                    )
                    nc.gpsimd.dma_start(
                        out=td8[:, npc * i : npc * (i + 1)],
                        in_=td8_d[:, npc * i : npc * (i + 1)],
                    )
                # y_all = cw1c.T @ ctx + cb1, per cloud -> y_nat [128cl, 4t, 128h]
                y_ps = psH.tile([128, 512], FP, tag="h1")
                for t in range(4):
                    nc.tensor.matmul(
                        y_ps[:, 128 * t : 128 * (t + 1)],
                        ctxT[:, 128 * t : 128 * (t + 1)], c1c,
                        start=True, stop=False, skip_group_check=True,
                    )
                    nc.tensor.matmul(
                        y_ps[:, 128 * t : 128 * (t + 1)], ones128, cb1r,
                        start=False, stop=True, skip_group_check=True,
                    )
                nc.scalar.activation(
                    y_nat[:, :, :].rearrange("p a b -> p (a b)"), y_ps[:, :], AF.Copy
                )

            def emit_yt2_piece(g):
                # yT2[j, 128*(16t+g) + h] = y_nat[8g + j, t, h]
                src = y_nat[8 * g : 8 * g + 8, :, :]
                dst = bass.AP(
                    tensor=yT2.tensor,
                    offset=yT2[0:8, :].offset + 128 * g,
                    ap=[list(yT2[0:8, :].ap[0]), [16 * 128, 4], [1, 128]],
                )
                nc.sync.dma_start(out=dst, in_=src)

            def emit_stage_a(k):
                """L1 + distance pipeline for chunk k"""
                p0 = CHUNK * k
                if k % 4 == 0:
                    nb = k // 4 + 1
                    if nb < NCH // 4:
                        posT4_cache[nb] = wpool.tile([128, 4, CHUNK], BF, tag="posT", name="posT4")
                        nc.sync.dma_start(
                            out=posT4_cache[nb][:, :, :].rearrange("p a b -> p (a b)"),
                            in_=posT_d[:, 4 * CHUNK * nb : 4 * CHUNK * (nb + 1)],
                        )
                    if k > 0:
                        del posT4_cache[k // 4 - 1]
                posT = posT4_cache[k // 4][:, k % 4, :]

                # corrections first (start=True), then gram accumulation
                gram = psG.tile([128, 256], FP, tag="gram")
                for c in range(4):
                    for half in range(2):
                        nc.tensor.matmul(
                            gram[64 * half : 64 * (half + 1), 64 * c : 64 * (c + 1)],
                            sel4[:, 64 * c : 64 * (c + 1)],
                            nsq4[0:4, 128 * k + 64 * half : 128 * k + 64 * (half + 1)],
                            start=True, stop=False, skip_group_check=True,
                            tile_position=(0, 64 * half),
                        )
                        nc.tensor.matmul(
                            gram[64 * half : 64 * (half + 1), 64 * c : 64 * (c + 1)],
                            posT[:, 128 * c + 64 * half : 128 * c + 64 * (half + 1)],
                            posT[:, 128 * c + 64 * half : 128 * c + 64 * (half + 1)],
                            start=False, stop=(c == 3), skip_group_check=True,
                            tile_position=(0, 64 * half),
                        )

                # ctx MLP layer 1: h1 = c1p.T @ posT + y (per cloud)
                h1 = psH.tile([128, 512], FP, tag="h1")
                nc.tensor.matmul(
                    h1[:, :], c1p, posT[:, :],
                    start=True, stop=False, skip_group_check=True,
                )
                nc.tensor.matmul(
                    h1[:, :], yT2[0:8, 128 * k : 128 * (k + 1)], bo8,
                    start=False, stop=True, skip_group_check=True,
                )

                # hier layer 1: g1 = h1p.T @ posT + h1dw (x) td + hb1
                # two column-halves stacked in partitions 0:32 / 32:64
                g1 = psQ.tile([64, 256], FP, tag="g1")
                q = k // 16
                o5 = CHUNK * (k % 16)
                for hf in range(2):
                    o2 = 256 * hf
                    nc.tensor.matmul(
                        g1[32 * hf : 32 * (hf + 1), :], h1p, posT[:, o2 : o2 + 256],
                        start=True, stop=False, skip_group_check=True,
                        tile_position=(0, 32 * hf),
                    )
                    nc.tensor.matmul(
                        g1[32 * hf : 32 * (hf + 1), :], h1de4[0:5, q, :],
                        td8[0:5, o5 + o2 : o5 + o2 + 256],
                        start=False, stop=True, skip_group_check=True,
                        tile_position=(0, 32 * hf),
                    )

                # distance path: top8 straight from gram PSUM, t1 stash on Pool
                wtop = spool.tile([128, 32], FP, tag="wtop")
                for c in range(4):
                    nc.vector.max(wtop[:, 8 * c : 8 * (c + 1)], gram[:, 64 * c : 64 * (c + 1)])
                nc.gpsimd.tensor_add(
                    t1s[:, 12 * k : 12 * (k + 1)].rearrange("p (c e) -> p c e", e=3),
                    wtop[:, :].rearrange("p (c e) -> p c e", e=8)[:, :, 1:4],
                    bcast(nsqn[:, 4 * k : 4 * (k + 1)], 3),
                )

                # psum->sbuf relu casts (PSUM eviction: Act/DVE only)
                h1T = xpool.tile([128, 512], BF, tag="h1T")
                nc.scalar.activation(h1T[:, :], h1[:, :], AF.Relu)
                g1Te = g1TeBufs[k % 3]
                nc.vector.tensor_scalar_max(g1Te[:, :], g1[:, :], 0.0)
                return h1T, g1Te

            def emit_stage_b(k, h1T, g1Te):
                """L2 + L3 + combine for chunk k"""
                z2c = psZ.tile([64, 512], FP, tag="z2c")
                nc.tensor.matmul(
                    z2c[:, :], c2w[:, :], h1T[:, :],
                    start=True, stop=True, skip_group_check=True,
                )
                zz = psM.tile([128, 68], FP, tag="zz")
                z2h = zz[:, 0:64].rearrange("p (a b) -> p a b", b=16)
                ctxn = zz[:, 64:68]
                for t in range(4):
                    if t < 2:
                        lhs, rhs2 = g1Te[0:32, 128 * t : 128 * (t + 1)], hw2
                    else:
                        lhs = g1Te[32:64, 128 * (t - 2) : 128 * (t - 1)]
                        rhs2 = hw2b
                    nc.tensor.matmul(
                        z2h[:, t, :], lhs, rhs2,
                        start=True, stop=False, skip_group_check=True,
                    )
                    nc.tensor.matmul(
                        z2h[:, t, :], ones128, hb2row,
                        start=False, stop=True, skip_group_check=True,
                    )
                stk = xpool.tile([64, 512], BF, tag="stk")
                nc.scalar.activation(stk[:, :], z2c[:, :], AF.Relu, bias=cb2)
                g2n = xpool.tile([128, 64], BF, tag="g2n")
                nc.scalar.activation(
                    g2n[:, :], z2h[:, :, :].rearrange("p a b -> p (a b)"), AF.Tanh
                )
                # L3 ctx (0.3*cw3 folded) straight to natural layout
                for t in range(4):
                    nc.tensor.matmul(
                        ctxn[:, t : t + 1], stk[:, 128 * t : 128 * (t + 1)], cw3b,
                        start=True, stop=True, skip_group_check=True,
                    )
                # L3 hier (0.2*hw3 folded): elementwise mul + reduce on DVE
                hm = spool.tile([128, 64], FP, tag="hm")
                nc.gpsimd.tensor_mul(hm[:, :], g2n[:, :], hw3rep)
                htmp = spool.tile([128, 4], FP, tag="htmp")
                nc.vector.reduce_sum(
                    htmp[:, :], hm[:, :].rearrange("p (c e) -> p c e", e=16), axis=AX.X
                )
                nc.vector.tensor_add(comb[:, 4 * k : 4 * (k + 1)], ctxn[:, :], htmp[:, :])

            def emit_tail():
                # d3 = sqrt(-2/9 * t1) for all chunks (single act-table switch)
                d3s = ppool.tile([128, NCH * 12], FP)
                nc.scalar.activation(d3s[:, :], t1s[:, :], AF.Sqrt, scale=-2.0 / 9.0)
                mds = ppool.tile([128, NCH * 4], FP)
                nc.vector.reduce_sum(
                    mds[:, :], d3s[:, :].rearrange("p (c e) -> p c e", e=3), axis=AX.X
                )
                nc.vector.tensor_scalar_add(mds[:, :], mds[:, :], 1e-6)
                dens = ppool.tile([128, NCH * 4], FP)
                nc.vector.reciprocal(dens[:, :], mds[:, :])
                # loc = tanh(dens) ~= dens*(1 - dens^2/3)  (|dens| < 0.1)
                s2 = ppool.tile([128, NCH * 4], FP)
                nc.vector.tensor_mul(s2[:, :], dens[:, :], dens[:, :])
                nc.vector.tensor_scalar(s2[:, :], s2[:, :], -1.0 / 3.0, 1.0, OP.mult, OP.add)
                loc = ppool.tile([128, NCH * 4], FP)
                nc.vector.tensor_mul(loc[:, :], s2[:, :], dens[:, :])
                # cf = comb - loc   (= combined - c*)
                cf = ppool.tile([128, NCH * 4], FP)
                nc.vector.scalar_tensor_tensor(
                    out=cf[:, :], in0=loc[:, :], scalar=-1.0, in1=comb[:, :],
                    op0=OP.mult, op1=OP.add,
                )
                # allreduce of per-slot sums
                red = ppool.tile([128, 1], FP)
                nc.vector.reduce_sum(red[:, :], cf[:, :], axis=AX.X)
                tfold = psZ.tile([1, 64], FP, tag="z2c")
                nc.tensor.matmul(
                    tfold[:, :], red[:, :], fold_sb,
                    start=True, stop=True, skip_group_check=True,
                )
                fold_out = ppool.tile([1, 64], FP)
                nc.vector.tensor_scalar_mul(fold_out[:, :], tfold[:, :], 1.0)
                nc.sync.dma_start(out=cc_in[:, :], in_=fold_out[:, :])
                if skip_collective:
                    # local bounce standing in for the collective (timing builds)
                    nc.sync.dma_start(out=cc_out[0:1, :], in_=cc_in[:, :])
                    nc.sync.dma_start(out=cc_out[1:2, :], in_=cc_in[:, :])
                    nc.sync.dma_start(out=cc_out[2:4, :], in_=cc_out[0:2, :])
                    nc.sync.dma_start(out=cc_out[4:8, :], in_=cc_out[0:4, :])
                else:
                    nc.gpsimd.collective_compute(
                        "AllGather", OP.bypass,
                        replica_groups=[list(range(NCORES))],
                        ins=[cc_in[:, :].opt()],
                        outs=[cc_out[:, :].opt()],
                    )
                for _x in range(extra_collectives):
                    nc.sync.dma_start(out=cc_in[:, :], in_=cc_out[0:1, :])
                    nc.gpsimd.collective_compute(
                        "AllGather", OP.bypass,
                        replica_groups=[list(range(NCORES))],
                        ins=[cc_in[:, :].opt()],
                        outs=[cc_out[:, :].opt()],
                    )
                gath2 = ppool.tile([8, 128], FP)
                nc.sync.dma_start(out=gath2[0:8, 0:64], in_=cc_out[:, :])
                nc.sync.dma_start(out=gath2[0:8, 64:128], in_=cc_out[:, :])
                tb = psG.tile([128, 1], FP, tag="gram")
                nc.tensor.matmul(
                    tb[:, :], gath2[:, :], ones81,
                    start=True, stop=True, skip_group_check=True,
                )
                b128 = ppool.tile([128, 1], FP)
                nc.vector.tensor_scalar_mul(b128[:, :], tb[:, :], 0.01 / TOTAL_CLOUDS)
                nc.vector.tensor_add(b128[:, :], b128[:, :], bconst[:, :])
                # smoothed = clip(0.9*cf + b128, -5, 2), natural layout out
                sm = ppool.tile([128, NCH * 4], FP)
                nc.vector.scalar_tensor_tensor(
                    out=sm[:, :], in0=cf[:, :], scalar=0.9, in1=bcast(b128[:, 0], NCH * 4),
                    op0=OP.mult, op1=OP.add,
                )
                nc.vector.tensor_scalar(sm[:, :], sm[:, :], 2.0, -5.0, OP.min, OP.max)
                nc.sync.dma_start(out=out_d[:, :], in_=sm[:, :])

            def emit_body():
                emit_prologue()
                hist = {}
                for k in range(NCH):
                    if k < 8:
                        emit_yt2_piece(2 * k)
                        emit_yt2_piece(2 * k + 1)
                    hist[k] = emit_stage_a(k)
                    if k >= 2:
                        emit_stage_b(k - 2, *hist.pop(k - 2))
                emit_stage_b(NCH - 2, *hist.pop(NCH - 2))
                emit_stage_b(NCH - 1, *hist.pop(NCH - 1))
                emit_tail()

            if repeat == 1:
                emit_body()
            else:
                with tc.For_i(0, repeat):
                    emit_body()

    nc.finalize()
    return nc


def pack_inputs(inputs):
    BT = 4 * 1024
    bf = ml_dtypes.bfloat16
    pf = np.asarray(inputs["positions"], np.float32).reshape(BT, P, D)
    ctx = np.asarray(inputs["context"], np.float32).reshape(BT, D)
    dep = np.asarray(inputs["hierarchy_depth"], np.float32).reshape(BT, P)

    cw1 = np.asarray(inputs["cw1"], np.float32)
    cb1 = np.asarray(inputs["cb1"], np.float32).reshape(-1)
    cw2 = np.asarray(inputs["cw2"], np.float32)
    cb2 = np.asarray(inputs["cb2"], np.float32).reshape(64, 1)
    cw3 = np.asarray(inputs["cw3"], np.float32).reshape(64, 1)
    cb3 = float(np.asarray(inputs["cb3"]).reshape(()))
    hw1 = np.asarray(inputs["hw1"], np.float32)
    hb1 = np.asarray(inputs["hb1"], np.float32).reshape(-1)
    hw2 = np.asarray(inputs["hw2"], np.float32)
    hb2 = np.asarray(inputs["hb2"], np.float32).reshape(1, 16)
    hw3 = np.asarray(inputs["hw3"], np.float32).reshape(-1)
    hb3 = float(np.asarray(inputs["hb3"]).reshape(()))
    mem = np.asarray(inputs["curvature_memory"], np.float32).reshape(-1)

    cstar = 0.3 * cb3 + 0.2 * hb3

    # h1de4[r, q, :] = hw1 depth row if r==q else (hb1 if r==4 else 0)
    h1de4 = np.zeros((5, 4, 32), np.float32)
    for q in range(4):
        h1de4[q, q, :] = hw1[128, :]
    h1de4[4, :, :] = hb1.reshape(1, 32)

    # packed weight blobs (one DMA each on device)
    wbig = np.zeros((128, 416), np.float32)
    wbig[:, 0:128] = cw1[128:256]          # c1c
    wbig[:, 128:256] = cw1[0:128]          # c1p
    wbig[:, 256:320] = cw2                 # c2w
    wbig[:, 320:352] = hw1[0:128]          # h1p
    wbig[:, 352:416] = np.tile(np.tile(0.2 * hw3, 4).reshape(1, 64), (128, 1))  # hw3rep
    wf32 = np.zeros((128, 385), np.float32)
    # nsqn filled per-core below
    wf32[:, 256:320] = np.tile(hb2.reshape(1, 16), (128, 4))   # hb2r
    wf32[:, 320:384] = np.tile(np.eye(64, dtype=np.float32), (2, 1))  # fold
    wf32[:, 384:385] = (0.09 * np.tile(mem, 2) + 0.91 * cstar).reshape(128, 1)  # bconst
    wsm = np.zeros((64, 1185), np.float32)
    wsm[0:8, 0:512] = np.kron(np.eye(8), np.ones((1, 64)))     # bo8
    wsm[0:5, 512:640] = h1de4.reshape(5, 128)                  # h1de4
    wsm[0:32, 640:656] = hw2
    wsm[32:64, 640:656] = hw2
    wsm[0:1, 1169:1185] = hb2                                  # hb2row
    wsm[0:64, 656:657] = 0.3 * cw3                             # cw3b
    wsm[0:1, 657:785] = cb1.reshape(1, 128)                    # cb1r
    wsm[0:4, 785:1041] = np.kron(np.eye(4), np.ones((1, 64)))  # sel4
    wsm[0:1, 1041:1169] = 1.0                                  # ones128
    wfs = np.zeros((64, 3), np.float32)
    wfs[0:64, 0:1] = cb2
    wfs[0:1, 1:2] = 1.0                                        # one11
    wfs[0:8, 2:3] = 1.0                                        # ones81

    weights = {
        "wbig": wbig.astype(bf),
        "wsm": wsm.astype(bf),
        "wfs": wfs,
    }

    in_maps = []
    for i in range(NCORES):
        sl = slice(i * NCLOUD, (i + 1) * NCLOUD)
        pos_bf = pf[sl].reshape(NPT, D).astype(bf)
        sq = np.einsum("nd,nd->n", pos_bf.astype(np.float32), pos_bf.astype(np.float32))
        nsq = (-0.5 * sq).astype(np.float32)
        td8 = np.empty((5, NPT // 4), np.float32)
        td8[0:4] = np.tanh(dep[sl].reshape(-1)).reshape(4, NPT // 4)
        td8[4] = 1.0
        # nsq4[c, 128k + 64*half + e] = nsq[512k + 128c + 64half + e]
        nsq4 = np.ascontiguousarray(
            nsq.reshape(NCH, 4, 2, 64).transpose(1, 0, 2, 3).reshape(4, NPT // 4)
        )
        wf32c = wf32.copy()
        wf32c[:, 0:256] = nsq.reshape(NCH, 4, 128).transpose(2, 0, 1).reshape(128, NCH * 4)
        m = {
            "posT": np.ascontiguousarray(pos_bf.T),
            "ctxT": np.ascontiguousarray(ctx[sl].astype(bf).T),
            "td8": td8.astype(bf),
            "nsq4": nsq4.astype(bf),
            "wf32": wf32c,
        }
        m.update(weights)
        in_maps.append(m)
    return in_maps


def unpack_out(res_results):
    full = []
    for r in res_results:
        o = np.asarray(r["out"], np.float32).reshape(128, NCH, 4)
        full.append(o.transpose(1, 2, 0).reshape(-1))
    return np.concatenate(full).reshape(4, 1024, P)


_NC_CACHE = None


def kernel(**inputs):
    global _NC_CACHE
    if _NC_CACHE is None:
        _NC_CACHE = build_bass()
    in_maps = pack_inputs(inputs)
    res = run_bass_kernel_spmd(_NC_CACHE, in_maps, core_ids=list(range(NCORES)))
    return unpack_out(res.results)
